# revision 14
# baseline (speedup 1.0000x reference)
"""Trainium2 Bass kernel for nn_DensityLoss (SPMD x8, row-sharded Gram).

Math
----
reference(centers, features, labels) depends only on centers X [C=4096,
D=256] (features unused; labels only via N=len(labels)=262144, a constant):

    sq_i = ||x_i||^2;  m = sum_i x_i;  S = sum sq;  q = sum sq^2
    Sigma = X'X;  w = sum_i sq_i x_i
    n_i  = C*sq_i + S - 2 x_i.m        (center_dist_i = n_i/(C-1); diag==0)
    sum n   = 2(C*S - m.m)
    sum n^2 = C^2 q + 3C S^2 + 4 m'Sigma m - 4C (w.m) - 4S (m.m)
    result  = (sum n)(C-1)^2 / (C * N * (sum n^2 - (sum n)^2/C))

Split
-----
Device (per core, a 512-row shard pre-cast to fp8e4m3 and pre-packed on the
host so each SBUF partition's bytes are contiguous in DRAM): the Gram blocks
    psA = Sigma[0:128, 0:256],  psB = Sigma[128:256, 128:256]
accumulated on PE with DoubleRow fp8 matmuls (two 128-row k-planes per
instruction) — ~98.5% of the required FLOPs — and written out as bf16.  The critical
path is just in-DMA -> matmul -> PSUM copy -> out-DMA; the in-DMA is split
across the sync and ACT HWDGE queues and the last tile-pair runs psB before
psA so the psB copy overlaps the final psA matmul.

Host (float64): the O(C*D) row stats (sq, S, q, m, w) from the f32 centers,
the sum of the 8 partial Grams, m'Sigma m, and the scalar formula.  Only
m'Sigma m — a ~0.05% term of the variance — carries fp8/bf16 error, so
overall rel err vs the f32 reference is ~2e-6.  tr(Sigma) is validated
against the exact host sum-of-squares per core (the first execution after
NEFF load has been observed to return corrupted PSUM copies; on mismatch we
re-run).
"""

import numpy as np

C, D, P = 4096, 256, 128
N_LABELS = 262144
# 4 cores, not 8: concurrent-core runs contend on shared resources on this
# part (measured: the same per-core chain is 3-4x slower with 8 concurrent
# cores than with 4), so 4 cores x 1024 rows beats 8 x 512 despite doubling
# the per-core DMA and matmul work.
N_CORES = 4
ROWS = C // N_CORES    # 1024 rows per core
NT = ROWS // P         # 8 tiles
GW = D + P             # 384 output cols: psA | psB
USE_FP8 = True

_CACHE = {}


def _build_nc(repeat=1, serial=False):
    """serial=True chains iteration r+1's in-DMA on iteration r's out-DMA
    completion, so the repeat slope measures full chain latency."""
    import concourse.bass as bass
    from concourse import mybir

    f32 = mybir.dt.float32
    bf16 = mybir.dt.bfloat16
    in_dt = mybir.dt.float8e4 if USE_FP8 else mybir.dt.bfloat16

    CH = NT // 2

    nc = bass.Bass()
    x_ext = nc.declare_dram_parameter("xb", [P, NT * D], in_dt, isOutput=False)
    out_ext = nc.declare_dram_parameter("out", [P, GW], bf16, isOutput=True)
    xv = x_ext[:, :].rearrange("p (t d) -> p t d", t=NT)

    from contextlib import ExitStack

    with ExitStack() as ctx:
        en = ctx.enter_context
        xh = en(nc.sbuf_tensor([P, NT, D], in_dt))
        ob = en(nc.sbuf_tensor([P, GW], bf16))
        psA = en(nc.psum_tensor([P, D], f32))
        psB = en(nc.psum_tensor([P, P], f32))
        s_d0 = en(nc.semaphore("s_d0"))
        s_d1 = en(nc.semaphore("s_d1"))
        s_ma = en(nc.semaphore("s_ma"))
        s_mb = en(nc.semaphore("s_mb"))
        s_oa = en(nc.semaphore("s_oa"))
        block = en(nc.Block())

        @block.sync
        def _(sync):
            for r in range(repeat):
                if serial and r > 0:
                    sync.wait_ge(s_oa, 32 * r)
                sync.dma_start(
                    out=xh[:, 0:CH, :], in_=xv[:, 0:CH, :]
                ).then_inc(s_d0, 16)

        @block.scalar
        def _(scalar):
            for r in range(repeat):
                if serial and r > 0:
                    scalar.wait_ge(s_oa, 32 * r)
                scalar.dma_start(
                    out=xh[:, CH:NT, :], in_=xv[:, CH:NT, :]
                ).then_inc(s_d1, 16)
                # psB finishes first (last tile-pair runs B then A), so its
                # copy + out-DMA overlap the last psA matmul and psA copy
                scalar.wait_ge(s_mb, r + 1)
                nc.scalar.copy(ob[:, D:GW], psB[:, :])
                scalar.dma_start(
                    out=out_ext[:, D:GW], in_=ob[:, D:GW]
                ).then_inc(s_oa, 16)
                scalar.wait_ge(s_ma, r + 1)
                nc.scalar.copy(ob[:, 0:D], psA[:, :])
                scalar.dma_start(
                    out=out_ext[:, 0:D], in_=ob[:, 0:D]
                ).then_inc(s_oa, 16)
            scalar.wait_ge(s_oa, 32 * repeat)

        @block.tensor
        def _(tensor):
            for r in range(repeat):
                if USE_FP8:
                    # DoubleRow: two 128-row k-planes per matmul; group g
                    # consumes tiles 2g, 2g+1 (chunk 0 holds tiles [0, CH))
                    for g in range(NT // 2):
                        tensor.wait_ge(s_d0 if 2 * g + 1 < CH else s_d1,
                                       16 * (r + 1))
                        first = g == 0
                        last = g == NT // 2 - 1
                        mmb = nc.tensor.matmul(
                            psB[:, :],
                            xh[:, 2 * g:2 * g + 2, P:D],
                            xh[:, 2 * g:2 * g + 2, P:D],
                            start=first, stop=last,
                            perf_mode=mybir.MatmulPerfMode.DoubleRow,
                        )
                        mma = nc.tensor.matmul(
                            psA[:, :],
                            xh[:, 2 * g:2 * g + 2, 0:P],
                            xh[:, 2 * g:2 * g + 2, 0:D],
                            start=first, stop=last,
                            perf_mode=mybir.MatmulPerfMode.DoubleRow,
                        )
                        if last:
                            mmb.then_inc(s_mb, 1)
                            mma.then_inc(s_ma, 1)
                else:
                    for t in range(NT):
                        tensor.wait_ge(s_d0 if t < CH else s_d1, 16 * (r + 1))
                        first = t == 0
                        last = t == NT - 1
                        mmb = nc.tensor.matmul(
                            psB[:, :], xh[:, t, P:D], xh[:, t, P:D],
                            start=first, stop=last,
                        )
                        mma = nc.tensor.matmul(
                            psA[:, :], xh[:, t, 0:P], xh[:, t, 0:D],
                            start=first, stop=last,
                        )
                        if last:
                            mmb.then_inc(s_mb, 1)
                            mma.then_inc(s_ma, 1)

    return nc


def _get_nc(repeat=1, serial=False):
    key = (repeat, serial, USE_FP8)
    if key not in _CACHE:
        _CACHE[key] = _build_nc(repeat, serial)
    return _CACHE[key]


def _pack_shard(shard_f32):
    """[512, 256] f32 -> [128, 4*256] fp8/bf16, partition-contiguous."""
    import ml_dtypes

    dt = ml_dtypes.float8_e4m3 if USE_FP8 else ml_dtypes.bfloat16
    xb = shard_f32.astype(dt)
    return np.ascontiguousarray(
        xb.reshape(NT, P, D).transpose(1, 0, 2).reshape(P, NT * D)
    )


def _host_combine(outs, x):
    """Sum per-core Gram blocks; evaluate the scalar formula in f64."""
    G = np.zeros((P, GW), dtype=np.float64)
    for o in outs:
        G += np.asarray(o, dtype=np.float64)
    B00 = G[:, 0:P]
    B01 = G[:, P:D]
    B11 = G[:, D:GW]

    xd = np.asarray(x, dtype=np.float64)
    sq = np.einsum("ij,ij->i", xd, xd)
    S = sq.sum()
    q = (sq * sq).sum()
    m = xd.sum(axis=0)
    w = sq @ xd

    mm = m @ m
    m0, m1 = m[:P], m[P:]
    mSm = m0 @ B00 @ m0 + 2.0 * (m0 @ B01 @ m1) + m1 @ B11 @ m1
    Wm = w @ m

    sum_n = 2.0 * (C * S - mm)
    sum_n2 = (
        C * C * q + 3.0 * C * S * S + 4.0 * mSm - 4.0 * C * Wm - 4.0 * S * mm
    )
    denom = sum_n2 - sum_n * sum_n / C
    result = sum_n * (C - 1.0) ** 2 / (C * N_LABELS * denom)
    return np.float32(result).reshape(())


def run(centers, trace=False):
    from concourse.bass_utils import run_bass_kernel_spmd

    x = np.ascontiguousarray(np.asarray(centers, dtype=np.float32))
    nc = _get_nc()
    in_maps = [
        {"xb": _pack_shard(x[i * ROWS:(i + 1) * ROWS])}
        for i in range(N_CORES)
    ]
    # exact per-shard sum-of-squares, for output validation
    sq_sh = [
        np.einsum(
            "ij,ij->",
            x[i * ROWS:(i + 1) * ROWS].astype(np.float64),
            x[i * ROWS:(i + 1) * ROWS].astype(np.float64),
        )
        for i in range(N_CORES)
    ]

    # The first execution after NEFF load returns corrupted PSUM copies
    # (observed: all-core Gram traces ~80% off, every later exec exact), so
    # validate tr(Sigma) per core against the host value and retry.
    r = None
    for _attempt in range(4):
        r = run_bass_kernel_spmd(
            nc, in_maps, core_ids=list(range(N_CORES)), trace=trace
        )
        ok = True
        for i, res in enumerate(r.results):
            o = np.asarray(res["out"], dtype=np.float64)
            tr = np.trace(o[:, 0:P]) + np.trace(o[:, D:GW])
            if not (abs(tr - sq_sh[i]) <= 0.01 * sq_sh[i]):
                ok = False
                break
        if ok:
            break
    out = _host_combine([res["out"] for res in r.results], x)
    return out, r


def kernel(centers, features=None, labels=None, **_):
    out, _r = run(centers)
    return out
